# revision 36
# baseline (speedup 1.0000x reference)
"""Block-sparse (top-2 block) attention for TRN2, sharded over 8 NeuronCores.

Problem: q [1,8,2048,64], k/v [1,8,4096,64], top2_idx [1,8,2048,2] over 64
key-blocks of 64 rows. Per query: softmax over the 128 keys of its 2 selected
blocks, weighted sum of V.

Strategy (per core = one head):
  - Route (query, slot) pairs into per-block groups of capacity 128 ("c-space",
    64 blocks x 128 slots). Each block then attends densely: its gathered
    queries vs its 64 keys.
  - One dma_gather pulls the 4096 routed q rows (+padding) into SBUF c-order.
  - PE pair-transposes build Qt (dk-major); per block-pair one PSUM tile holds
    scores^T for both blocks; ACT exp (no max-subtraction needed: scores are
    ~N(0,1)); second matmul against [V | 1] gives numerator rows and the
    softmax denominator in one shot.
  - Partial rows go to a DRAM bounce; one dma_gather brings each query's two
    partials back side-by-side; DVE combines and normalizes.
Routing indices are computed on host (tiny: argsort of 4096 block ids/head)
and passed as int16 index tensors in the dma_gather wire format.
"""

import numpy as np

import concourse.bass as bass
import concourse.bacc as bacc
import concourse.mybir as mybir
import concourse.tile as tile
from concourse.masks import make_identity

F32 = mybir.dt.float32
I16 = mybir.dt.int16

T_Q = 2048
T_K = 4096
DK = 64
BS = 64
NB = T_K // BS        # 64 key blocks
CAP = 128             # routed-query capacity per block
NG = NB * CAP         # 8192 gather slots
NR = 2 * T_Q          # 4096 real (query, slot) refs
NPAIR = NB // 2
OROW = 128            # padded opart row: 65 useful fp32 -> 512B stride
N_CORES = 8

EXP = mybir.ActivationFunctionType.Exp
ADD = mybir.AluOpType.add
MULT = mybir.AluOpType.mult


def build_module(reps: int = 1, variant: str = "full",
                 gather_queues: int = 1) -> bass.Bass:
    """reps>1 repeats the whole body back-to-back (for wall-clock timing of
    the steady-state per-iteration cost; pools make iterations overlap the
    same way a stream of real invocations would)."""
    nc = bacc.Bacc("TRN2", target_bir_lowering=False, debug=False,
                   num_swdge_queues=max(1, gather_queues))
    q = nc.dram_tensor("q", [T_Q, DK], F32, kind="ExternalInput")
    k = nc.dram_tensor("k", [T_K, DK], F32, kind="ExternalInput")
    v = nc.dram_tensor("v", [T_K, DK], F32, kind="ExternalInput")
    qg_idx = nc.dram_tensor("qg_idx", [128, NG // 16], I16, kind="ExternalInput")
    og_idx = nc.dram_tensor("og_idx", [128, NR // 16], I16, kind="ExternalInput")
    out = nc.dram_tensor("out", [T_Q, DK], F32, kind="ExternalOutput")

    with tile.TileContext(nc) as tc:
        with (
            tc.tile_pool(name="const", bufs=1) as constp,
            tc.tile_pool(name="kv", bufs=1) as kvp,
            tc.tile_pool(name="big", bufs=2 if reps > 1 else 1) as bigp,
            tc.tile_pool(name="work", bufs=8) as workp,
            tc.tile_pool(name="psum", bufs=2, space="PSUM") as psump,
            tc.tile_pool(name="psum_o", bufs=2, space="PSUM") as psumop,
            tc.tile_pool(name="dram", bufs=2 if reps > 1 else 1,
                         space="DRAM") as dramp,
        ):
            ident = constp.tile([128, 128], F32)
            make_identity(nc, ident[:])

            qgi = constp.tile([128, NG // 16], I16)
            nc.sync.dma_start(out=qgi[:], in_=qg_idx[:])
            ogi = constp.tile([128, NR // 16], I16)
            nc.sync.dma_start(out=ogi[:], in_=og_idx[:])

            def body():
                _emit_body(nc, tc, q, k, v, out, ident, qgi, ogi,
                           kvp, bigp, workp, psump, psumop, dramp,
                           variant, gather_queues)

            if reps == 1:
                body()
            else:
                with tc.For_i(0, reps, 1):
                    body()

    nc.compile()
    return nc


def _emit_body(nc, tc, q, k, v, out, ident, qgi, ogi,
               kvp, bigp, workp, psump, psumop, dramp,
               variant="full", gather_queues=1, psumkp=None):
    if True:
        if True:
            # Per block j: K_j^T (dk x keys) at partitions 0:64, and V_j with an
            # appended ones column, also at partitions 0:64 (PSUM outputs and
            # matmul operands must share base partition 0/0 or 64/64; keeping
            # everything per-block at base 0 satisfies the walrus verifier).
            kts = []
            vhats = []
            for m in range(NPAIR):
                kchunk = workp.tile([128, DK], F32, tag="kchunk")
                nc.sync.dma_start(out=kchunk[:], in_=k[m * 128:(m + 1) * 128, :])
                for par in range(2):
                    j = 2 * m + par
                    ktps = (psumkp or psump).tile([64, DK], F32, tag="kt_ps")
                    nc.tensor.transpose(
                        out=ktps[:],
                        in_=kchunk[par * 64:(par + 1) * 64, :],
                        identity=ident[par * 64:(par + 1) * 64,
                                       par * 64:(par + 1) * 64],
                    )
                    ktp = kvp.tile([64, DK], F32, tag=f"ktp{j}")
                    nc.vector.tensor_copy(out=ktp[:], in_=ktps[:])
                    kts.append(ktp)

                    vhat = kvp.tile([64, DK + 1], F32, tag=f"vhat{j}")
                    nc.sync.dma_start(
                        out=vhat[:, 0:DK], in_=v[j * BS:(j + 1) * BS, :]
                    )
                    nc.vector.memset(vhat[:, DK:DK + 1], 1.0)
                    vhats.append(vhat)

            # Routed q rows: slot g=(block*128+rank) -> partition g%128, chunk g//128.
            # Chunk j == block j since CAP == 128. Split into one tile per
            # queue so downstream blocks start as soon as their part lands.
            nq = gather_queues
            per = NG // nq               # slots per queue (multiple of 128)
            qg_parts = [
                bigp.tile([128, per // 128, DK], F32, tag=f"qg{qq}",
                          name=f"qg{qq}")
                for qq in range(nq)
            ]

            def qg_chunk(j):
                return qg_parts[(j * 128) // per][:, j - (per // 128) * ((j * 128) // per), :]

            if variant == "nogather":
                for qq in range(nq):
                    nc.sync.dma_start(
                        out=qg_parts[qq][:],
                        in_=q[:].rearrange("(c p) d -> p c d", p=128)[
                            :, 0:per // 128, :],
                    )
            elif variant == "scatterq":
                # Route q by scatter-add (4096 descriptors instead of 8192):
                # zero Qg, scatter q rows (idxs = og_idx, dest g per real ref),
                # read back block-aligned.
                qgd = dramp.tile([NG, DK], F32, tag="qgd")
                zt = workp.tile([128, 1024], F32, tag="zt")
                nc.vector.memset(zt[:], 0.0)
                for zz in range(4):
                    nc.sync.dma_start(
                        out=qgd[zz * (NG // 4):(zz + 1) * (NG // 4), :]
                        .rearrange("(c p) d -> p c d", p=128),
                        in_=zt[:].rearrange("p (c d) -> p c d", c=16),
                    )
                qdup = workp.tile([128, 32, DK], F32, tag="qdup")
                for ss in range(2):
                    nc.sync.dma_start(
                        out=qdup[:, 16 * ss:16 * (ss + 1), :],
                        in_=q[:].rearrange("(c p) d -> p c d", p=128),
                    )
                nper = NR // nq
                for qq in range(nq):
                    nc.gpsimd.dma_scatter_add(
                        out_ap=qgd[:],
                        in_ap=qdup[:, (nper // 128) * qq:(nper // 128) * (qq + 1), :],
                        idxs_ap=ogi[:, (nper // 16) * qq:(nper // 16) * (qq + 1)],
                        num_idxs=nper,
                        num_idxs_reg=nper,
                        elem_size=DK,
                        single_packet=False,
                        queue_num=qq,
                    )
                for qq in range(nq):
                    nc.sync.dma_start(
                        out=qg_parts[qq][:],
                        in_=qgd[per * qq:per * (qq + 1), :]
                        .rearrange("(c p) d -> p c d", p=128),
                    )
            else:
                for qq in range(nq):
                    nc.gpsimd.dma_gather(
                        out_ap=qg_parts[qq][:],
                        in_ap=q[:],
                        idxs_ap=qgi[:, (per // 16) * qq:(per // 16) * (qq + 1)],
                        num_idxs=per,
                        num_idxs_reg=per,
                        elem_size=DK,
                        single_packet=False,
                        queue_num=qq,
                    )

            opart = dramp.tile([NG, OROW], F32)

            W = DK + 1
            if variant == "unpaired":
                for j in range(NB):
                    qt_ps = psump.tile([64, 128], F32, tag="qt_ps")
                    nc.tensor.transpose(
                        out=qt_ps[:], in_=qg_chunk(j), identity=ident[:]
                    )
                    qt_sb = workp.tile([64, 128], F32, tag="qt_sb")
                    nc.vector.tensor_copy(out=qt_sb[:], in_=qt_ps[:])
                    s_ps = psump.tile([64, 128], F32, tag="s_ps")
                    nc.tensor.matmul(
                        s_ps[:], lhsT=kts[j][:], rhs=qt_sb[:],
                        start=True, stop=True,
                    )
                    e_sb = workp.tile([64, 128], F32, tag="e_sb")
                    nc.scalar.activation(
                        out=e_sb[:], in_=s_ps[:], func=EXP, scale=1.0 / np.sqrt(DK)
                    )
                    o_ps = psumop.tile([128, W], F32, tag="o_ps")
                    nc.tensor.matmul(
                        o_ps[:], lhsT=e_sb[:], rhs=vhats[j][:],
                        start=True, stop=True,
                    )
                    o_sb = workp.tile([128, W], F32, tag="o_sb")
                    nc.scalar.copy(out=o_sb[:], in_=o_ps[:])
                    nc.sync.dma_start(
                        out=opart[j * CAP:(j + 1) * CAP, 0:W], in_=o_sb[:]
                    )
            for m in range(NPAIR if variant in ("full", "nogather", "scatterq")
                           else 0):
                j0, j1 = 2 * m, 2 * m + 1
                # Qt for both blocks side by side: [64 dk, 256]
                qt_ps = psump.tile([64, 256], F32, tag="qt_ps")
                nc.tensor.transpose(
                    out=qt_ps[:, 0:128], in_=qg_chunk(j0), identity=ident[:]
                )
                nc.tensor.transpose(
                    out=qt_ps[:, 128:256], in_=qg_chunk(j1), identity=ident[:]
                )
                qt_sb = workp.tile([64, 256], F32, tag="qt_sb")
                nc.vector.tensor_copy(out=qt_sb[:], in_=qt_ps[:])

                # scores^T [64 keys, 128 queries] per block, shared PSUM tile
                s_ps = psump.tile([64, 256], F32, tag="s_ps")
                nc.tensor.matmul(
                    s_ps[:, 0:128], lhsT=kts[j0][:], rhs=qt_sb[:, 0:128],
                    start=True, stop=True,
                )
                nc.tensor.matmul(
                    s_ps[:, 128:256], lhsT=kts[j1][:], rhs=qt_sb[:, 128:256],
                    start=True, stop=True,
                )

                e_sb = workp.tile([64, 256], F32, tag="e_sb")
                nc.scalar.activation(
                    out=e_sb[:], in_=s_ps[:], func=EXP, scale=1.0 / np.sqrt(DK)
                )

                # numerator rows + denominator: [o | Z] per gathered query.
                o_ps = psumop.tile([128, 2 * W], F32, tag="o_ps")
                nc.tensor.matmul(
                    o_ps[:, 0:W], lhsT=e_sb[:, 0:128], rhs=vhats[j0][:],
                    start=True, stop=True,
                )
                nc.tensor.matmul(
                    o_ps[:, W:2 * W], lhsT=e_sb[:, 128:256], rhs=vhats[j1][:],
                    start=True, stop=True,
                )
                o_sb = workp.tile([128, 2 * W], F32, tag="o_sb")
                nc.scalar.copy(out=o_sb[:], in_=o_ps[:])

                nc.sync.dma_start(
                    out=opart[j0 * CAP:(j1 + 1) * CAP, 0:W].rearrange(
                        "(g p) x -> p g x", p=128
                    ),
                    in_=o_sb[:].rearrange("p (g x) -> p g x", g=2),
                )

            # Pull each query's two partial rows back: dest slot r = s*2048 + t.
            og_all = bigp.tile([128, 32, OROW], F32)
            if variant == "nogather":
                nc.sync.dma_start(
                    out=og_all[:],
                    in_=opart[0:NR, :].rearrange("(c p) d -> p c d", p=128),
                )
            else:
                nq = gather_queues
                per = NR // nq
                for qq in range(nq):
                    nc.gpsimd.dma_gather(
                        out_ap=og_all[:, (per // 128) * qq:(per // 128) * (qq + 1), :],
                        in_ap=opart[:],
                        idxs_ap=ogi[:, (per // 16) * qq:(per // 16) * (qq + 1)],
                        num_idxs=per,
                        num_idxs_reg=per,
                        elem_size=OROW,
                        single_packet=False,
                        queue_num=qq,
                    )

            osum = bigp.tile([128, 16, 65], F32)
            nc.vector.tensor_tensor(
                out=osum[:], in0=og_all[:, 0:16, 0:65], in1=og_all[:, 16:32, 0:65],
                op=ADD,
            )
            zrec = bigp.tile([128, 16], F32)
            nc.vector.reciprocal(out=zrec[:], in_=osum[:, :, 64])
            outv = bigp.tile([128, 16, DK], F32)
            nc.vector.tensor_tensor(
                out=outv[:],
                in0=osum[:, :, 0:DK],
                in1=zrec[:, :, None].to_broadcast([128, 16, DK]),
                op=MULT,
            )
            nc.sync.dma_start(
                out=out[:].rearrange("(c p) d -> p c d", p=128), in_=outv[:]
            )


_CACHE: dict = {}


def get_module() -> bass.Bass:
    if "m" not in _CACHE:
        _CACHE["m"] = build_module(gather_queues=4)
    return _CACHE["m"]


def routing(idx2: np.ndarray) -> tuple[np.ndarray, np.ndarray]:
    """idx2: [2048, 2] int32 block ids -> (qg_idx [128,512], og_idx [128,256])
    int16 index tensors in dma_gather wire format (flat n at [n%16, n//16])."""
    blocks = np.concatenate([idx2[:, 0], idx2[:, 1]]).astype(np.int64)
    counts = np.bincount(blocks, minlength=NB)
    if counts.max() > CAP:
        raise ValueError(f"block over capacity: {counts.max()} > {CAP}")
    order = np.argsort(blocks, kind="stable")
    starts = np.cumsum(counts) - counts
    within = np.arange(NR) - np.repeat(starts, counts)
    g = np.empty(NR, np.int64)
    g[order] = blocks[order] * CAP + within
    tinv = np.zeros(NG, np.int16)          # unused slots gather q[0] harmlessly
    tinv[g] = (np.arange(NR) % T_Q).astype(np.int16)
    # Wire format: flat index n at [n%16, n//16], replicated to all 8 GPSIMD
    # core groups (each Q7 core reads its own 16-partition window).
    qg = np.empty((128, NG // 16), np.int16)
    qg[:] = np.tile(tinv.reshape(NG // 16, 16).T, (8, 1))
    og = np.empty((128, NR // 16), np.int16)
    og[:] = np.tile(g.astype(np.int16).reshape(NR // 16, 16).T, (8, 1))
    return qg, og


def make_in_maps(q, k, v, top2_idx):
    in_maps = []
    for i in range(N_CORES):
        qg, og = routing(np.asarray(top2_idx[0, i]))
        in_maps.append({
            "q": np.ascontiguousarray(np.asarray(q[0, i], dtype=np.float32)),
            "k": np.ascontiguousarray(np.asarray(k[0, i], dtype=np.float32)),
            "v": np.ascontiguousarray(np.asarray(v[0, i], dtype=np.float32)),
            "qg_idx": qg,
            "og_idx": og,
        })
    return in_maps


def kernel(**inputs) -> np.ndarray:
    q = np.asarray(inputs["q"])
    k = np.asarray(inputs["k"])
    v = np.asarray(inputs["v"])
    top2_idx = np.asarray(inputs["top2_idx"])
    assert int(inputs["BS"]) == BS
    assert q.shape == (1, N_CORES, T_Q, DK), q.shape
    assert k.shape == (1, N_CORES, T_K, DK), k.shape

    from concourse.bass_utils import run_bass_kernel_spmd

    nc = get_module()
    in_maps = make_in_maps(q, k, v, top2_idx)
    res = run_bass_kernel_spmd(nc, in_maps, list(range(N_CORES)))
    out = np.stack([res.results[i]["out"] for i in range(N_CORES)])
    return out[None].astype(np.float32)
